# revision 1
# baseline (speedup 1.0000x reference)
"""Cartesian-product expansion kernel for Trainium2 (8 NeuronCores).

reference:
    a = repeat(emb_in, n, axis=0)       # [n*n, f]   a[k] = emb_in[k // n]
    b = tile(emb_in, (n, 1))            # [n*n, f]   b[k] = emb_in[k % n]
    w = tile(sum_weights[:, None], (n, 1))
    out = concat([a, b, w], axis=1)     # [n*n, 2f+1]

Pure data movement; the 1.09 GB f32 output is HBM-write-bound.

Sharding: row-block over i = k // n. Core c owns i in [c*256, (c+1)*256).

Per-core layout trick: for a fixed i, the output block [n, 65] has columns
32:65 (b|w) identical for every i. We keep K persistent SBUF buffers whose
b|w columns are written once; per iteration only the 32 "a" columns are
re-broadcast (DVE), then the whole buffer is DMA'd to DRAM contiguously.
j-rows map to partitions as j = p*16 + t so each partition emits one
contiguous DRAM segment per i-block (4160 B) — large-descriptor, full-rate
HBM writes.
"""

import numpy as np

N = 2048          # rows of emb_in
F = 32            # features
ROW = 2 * F + 1   # 65 output columns
P = 128           # SBUF partitions
NCORES = 8
IPC = N // NCORES  # 256 i-values per core
T = N // P         # 16 j-rows per partition
G = 8              # i-blocks per DMA
K = 3              # pipeline depth (persistent template buffers)
ITERS = IPC // G   # 32

_NC = None


def _build():
    global _NC
    if _NC is not None:
        return _NC
    import concourse.bass as bass  # noqa: F401
    import concourse.bacc as bacc
    import concourse.tile as tile
    from concourse import mybir

    f32 = mybir.dt.float32
    nc = bacc.Bacc("TRN2", target_bir_lowering=False, debug=False,
                   num_devices=NCORES)

    emb_slice = nc.dram_tensor("emb_slice", [IPC, F], f32, kind="ExternalInput")
    emb_full = nc.dram_tensor("emb_full", [N, F], f32, kind="ExternalInput")
    sw = nc.dram_tensor("sw", [N], f32, kind="ExternalInput")
    out = nc.dram_tensor("out", [IPC * N, ROW], f32, kind="ExternalOutput")

    # out rows: r = ((it*G + g)*P + p)*T + t ; view [it][p][g][(t c)]
    O = out[:].rearrange("(it g p t) c -> it p g (t c)", g=G, p=P, t=T)

    with tile.TileContext(nc) as tc:
        with tc.tile_pool(name="singles", bufs=1) as singles:
            # emb_slice replicated to every partition: es[p, i, :] = emb_slice[i]
            es = singles.tile([P, IPC, F], f32, tag="es")
            nc.gpsimd.dma_start(
                out=es[:],
                in_=emb_slice[:].unsqueeze(0).broadcast_to((P, IPC, F)),
            )
            # emb_full rows for this partition's j-rows: e[p, t, :] = emb_full[p*T + t]
            e = singles.tile([P, T, F], f32, tag="e")
            nc.sync.dma_start(out=e[:], in_=emb_full[:].rearrange("(p t) f -> p t f", p=P))
            swt = singles.tile([P, T], f32, tag="swt")
            nc.sync.dma_start(out=swt[:], in_=sw[:].rearrange("(p t) -> p t", p=P))

            # K persistent buffers; b|w columns filled once.
            bufs = []
            for k in range(K):
                tk = singles.tile([P, G, T * ROW], f32, tag=f"buf{k}")
                tkv = tk[:].rearrange("p g (t c) -> p g t c", t=T)
                nc.vector.tensor_copy(
                    tkv[:, :, :, F:2 * F],
                    e[:].unsqueeze(1).broadcast_to((P, G, T, F)),
                )
                nc.vector.tensor_copy(
                    tkv[:, :, :, 2 * F:ROW],
                    swt[:].unsqueeze(1).unsqueeze(3).broadcast_to((P, G, T, 1)),
                )
                bufs.append(tk)

            for it in range(ITERS):
                tk = bufs[it % K]
                tkv = tk[:].rearrange("p g (t c) -> p g t c", t=T)
                nc.vector.tensor_copy(
                    tkv[:, :, :, 0:F],
                    es[:, it * G:(it + 1) * G, :].unsqueeze(2).broadcast_to((P, G, T, F)),
                )
                nc.sync.dma_start(out=O[it], in_=tk[:])

    nc.compile()
    _NC = nc
    return nc


def kernel(emb_in, sum_weights, _profile=False):
    from concourse.bass_utils import run_bass_kernel_spmd

    nc = _build()
    emb_in = np.ascontiguousarray(np.asarray(emb_in, dtype=np.float32))
    sum_weights = np.ascontiguousarray(np.asarray(sum_weights, dtype=np.float32))
    assert emb_in.shape == (N, F) and sum_weights.shape == (N,)

    in_maps = [
        {
            "emb_slice": emb_in[c * IPC:(c + 1) * IPC],
            "emb_full": emb_in,
            "sw": sum_weights,
        }
        for c in range(NCORES)
    ]
    res = run_bass_kernel_spmd(nc, in_maps, list(range(NCORES)), trace=_profile)
    full = np.concatenate([res.results[c]["out"] for c in range(NCORES)], axis=0)
    if _profile:
        return full, res.exec_time_ns
    return full
